# revision 3
# baseline (speedup 1.0000x reference)
"""Corr1d (stereo cost volume) Trainium2 kernel, v3.

corrmap[b, i, h, w] = sum_c fL[b, c, h, w] * fR[b, c, h, w - i],  i in [0, 64)
Shapes: fL, fR [8, 128, 160, 320] f32 -> corrmap [8, 64, 160, 320] f32.
Sharding: data-parallel over batch; core k handles batch element k.
Host: inputs cast f32->bf16 (RTNE) before upload; output computed in bf16 on
device and upcast f32 on host.

v3 changes vs v2 (256us baseline):
  * NH=32 super-batches (5 instead of 10) -> half the DMA count per row.
  * Dump-side per-tile address shift: scratch row pitch GP2 with each w-tile
    q's data shifted by -256*q elements.  Readback address then becomes
    affine in the partition index p (p*(GP2-8) + hc2*768 + i*8 + h8), so one
    3-dim readback per group covers all hc2 chunks: 3 readbacks per super
    (vs 12 equivalent), each [128, 2048].
  * Ring rebalance: fL loads on gpsimd (SWDGE), fR loads on scalar HWDGE,
    dumps+readbacks on sync HWDGE (FIFO order guarantees dump->readback
    DRAM ordering), stores split gpsimd/scalar/sync.

Pipeline per super-batch (32 h rows):
  1. Load fL/fR rows to SBUF (bf16), fR with 64 zero-pad cols at the start.
  2. Band matmuls M=32 col-tiled 4x as in v2 (hq in [0,8)).
  3. PSUM -> band tiles b01/b2 in h8-interleaved layout, memset i>w zones.
  4. 3 dumps to DRAM scratch (per-tile -256 shift), 3 diagonal readbacks
     T[p, hc2*512 + i*8 + h8].
  5. TensorE-transpose T chunks -> U[(i16,h8), w] in PSUM, copy to stg,
     16 stores out[i, h, w] (AP [[HW,16],[W,8],[1,W]]).
  Stages software-pipelined one super deep.

Self-contained: shapes hardcoded; requires only numpy + ml_dtypes + concourse.
"""

import ml_dtypes
import numpy as np

import concourse.bacc as bacc
import concourse.bass as bass
import concourse.mybir as mybir
from concourse.bass_utils import run_bass_kernel_spmd
from concourse.tile import TileContext
from concourse.masks import make_identity

F32 = mybir.dt.float32
BF16 = mybir.dt.bfloat16

N_CORES = 8
C = 128           # channels (matmul contraction dim)
H = 160
W = 320
D = 64            # disparities
NH = 32           # h rows per super-batch
NB = H // NH      # super-batches (5)
NS = 95           # band columns per 32-wide w-tile (32 + 63)
SC = 96           # stored band columns (n dim) per tile row
FRPAD = 64        # zero pad columns at the start of the fR buffer
HW = H * W
NC2 = NH // 8     # hc2 chunks per super (4)
ROW = SC * 8      # 768: scratch row (n, h8) elements per (p, hc)
GP2 = (H // 8) * ROW + ROW   # 16128: scratch row pitch (+768 shift margin)
GG = 128 * GP2    # scratch elements per group

_cache = {}


def _build():
    nc = bacc.Bacc("TRN2", target_bir_lowering=False, debug=False,
                   num_devices=N_CORES)
    fL = nc.dram_tensor("fL", [C, H, W], BF16, kind="ExternalInput")
    fR = nc.dram_tensor("fR", [C, H, W], BF16, kind="ExternalInput")
    out = nc.dram_tensor("out", [D, H, W], BF16, kind="ExternalOutput")
    scratch = nc.dram_tensor("scratch", [3, 128, GP2], BF16)

    with TileContext(nc) as tc:
        fLb = [nc.alloc_sbuf_tensor(f"fLb{i}", [C, NH * W], BF16)
               for i in range(2)]
        fRb = [nc.alloc_sbuf_tensor(f"fRb{i}", [C, FRPAD + NH * W], BF16)
               for i in range(2)]
        ident = nc.alloc_sbuf_tensor("ident", [128, 128], BF16)
        make_identity(nc, ident.ap())
        for i in range(2):
            nc.vector.memset(fRb[i].ap()[:, 0:FRPAD], 0.0)

        with (
            tc.tile_pool(name="sb", bufs=2) as pool,
            tc.tile_pool(name="ps", bufs=2, space="PSUM") as pp,
        ):
            def emit_loads(b):
                li, ri = fLb[b % 2], fRb[b % 2]
                h0 = b * NH
                nc.gpsimd.dma_start(
                    out=li.ap(),
                    in_=bass.AP(fL, h0 * W, [[HW, C], [1, NH * W]]),
                )
                nc.scalar.dma_start(
                    out=ri.ap()[:, FRPAD:],
                    in_=bass.AP(fR, h0 * W, [[HW, C], [1, NH * W]]),
                )

            def emit_front(b):
                # matmuls + psum->band copies + garbage memsets + dumps +
                # readbacks for super-batch b; returns the T tiles.
                li, ri = fLb[b % 2], fRb[b % 2]
                # b01: [128, (g01, hc2, n, h8)]; b2: [64, (hc2, n, h8)]
                b01 = pool.tile([128, 2 * NC2 * ROW], BF16, tag="b01",
                                name=f"b01_{b}")
                b2 = pool.tile([64, NC2 * ROW], BF16, tag="b2", name=f"b2_{b}")
                for hq in range(NH // 4):
                    # 4 h rows share one PSUM bank (4*95 f32 = 1520B);
                    # copy writes 8B-coalesced (n, delta4) runs into the
                    # h8-interleaved band layout.
                    hc2, h8b = hq // 2, 4 * (hq % 2)
                    pss = []
                    for g in range(3):
                        P = 64 if g == 2 else 128
                        nt = 2 if g == 2 else 4
                        ps = pp.tile([P, 4 * NS], F32, tag=f"ps{g}",
                                     name=f"ps{g}_{b}_{hq}")
                        pss.append(ps)
                        for j4 in range(4):
                            hh = 4 * hq + j4
                            for q in range(nt):
                                wt = g * 128 + 32 * q
                                lhsT = bass.AP(li, hh * W + wt,
                                               [[NH * W, C], [1, 32]])
                                rhs = bass.AP(ri, FRPAD + hh * W + wt + 31,
                                              [[FRPAD + NH * W, C], [-1, NS]])
                                nc.tensor.matmul(
                                    ps[32 * q:32 * q + 32,
                                       j4 * NS:(j4 + 1) * NS],
                                    lhsT, rhs, start=True, stop=True,
                                    tile_position=(0, 32 * q),
                                )
                    for g in range(3):
                        P = 64 if g == 2 else 128
                        tile = b2 if g == 2 else b01
                        base = ((g % 2) * NC2 * ROW if g < 2 else 0) \
                            + hc2 * ROW + h8b
                        pitch = tile.tensor.shape[-1]
                        o = bass.AP(tile.tensor, base,
                                    [[pitch, P], [8, NS], [1, 4]])
                        i_ = bass.AP(pss[g].tensor, 0,
                                     [[4 * NS, P], [1, NS], [NS, 4]])
                        if (hq + g) % 2 == 0:
                            nc.vector.tensor_copy(out=o, in_=i_)
                        else:
                            nc.scalar.copy(o, i_)
                # zero i > w zones (w-tiles 0 and 1): band cols n>=32 / n>=64
                nc.vector.memset(
                    bass.AP(b01.tensor, 32 * 8,
                            [[2 * NC2 * ROW, 32], [ROW, NC2],
                             [1, (SC - 32) * 8]]),
                    0.0)
                nc.vector.memset(
                    bass.AP(b01.tensor, 32 * (2 * NC2 * ROW) + 64 * 8,
                            [[2 * NC2 * ROW, 32], [ROW, NC2],
                             [1, (SC - 64) * 8]]),
                    0.0)

                # dumps: per-tile -256 shift; super b lands at hc2 offset
                # b*NC2*ROW within each scratch row.
                for g in range(2):
                    nc.sync.dma_start(
                        out=bass.AP(scratch, g * GG + b * NC2 * ROW,
                                    [[32 * GP2 - 256, 4], [GP2, 32],
                                     [1, NC2 * ROW]]),
                        in_=bass.AP(b01.tensor, g * NC2 * ROW,
                                    [[2 * NC2 * ROW, 128], [1, NC2 * ROW]]),
                    )
                nc.sync.dma_start(
                    out=bass.AP(scratch, 2 * GG + b * NC2 * ROW,
                                [[32 * GP2 - 256, 2], [GP2, 32],
                                 [1, NC2 * ROW]]),
                    in_=b2[:, :],
                )
                # diagonal readbacks: T[p, hc2*512 + i*8 + h8]
                # addr = p*(GP2-8) + 248 + hc2*768 + i*8 + h8  (affine in p)
                Ts = []
                for g in range(3):
                    P = 64 if g == 2 else 128
                    T = pool.tile([P, NC2 * 512], BF16, tag=f"T{g}",
                                  name=f"T{g}_{b}")
                    Ts.append(T)
                    nc.sync.dma_start(
                        out=T[:, :],
                        in_=bass.AP(scratch,
                                    g * GG + b * NC2 * ROW + 31 * 8,
                                    [[GP2 - 8, P], [ROW, NC2], [1, 512]]),
                    )
                return Ts

            def emit_back(b, Ts):
                # transposes + staging copies + output DMAs for super b
                stg = pool.tile([128, NC2 * 4 * W], BF16, tag="stg",
                                name=f"stg_{b}")
                for hc2 in range(NC2):
                    for a in range(4):
                        u = pp.tile([128, W], BF16, tag="U",
                                    name=f"U_{b}_{hc2}_{a}")
                        cs = hc2 * 512 + 128 * a
                        nc.tensor.transpose(
                            u[:, 0:128], Ts[0][:, cs:cs + 128], ident.ap())
                        nc.tensor.transpose(
                            u[:, 128:256], Ts[1][:, cs:cs + 128], ident.ap())
                        nc.tensor.transpose(
                            u[:, 256:320], Ts[2][:, cs:cs + 128],
                            ident.ap()[0:64, 0:64])
                        o = stg[:, (hc2 * 4 + a) * W:(hc2 * 4 + a + 1) * W]
                        if a % 2 == 0:
                            nc.vector.tensor_copy(out=o, in_=u[:, :])
                        else:
                            nc.scalar.copy(o, u[:, :])
                for hc2 in range(NC2):
                    for a in range(4):
                        k = hc2 * 4 + a
                        eng = (nc.sync, nc.scalar, nc.gpsimd)[k % 3]
                        eng.dma_start(
                            out=bass.AP(out,
                                        16 * a * HW + (NH * b + 8 * hc2) * W,
                                        [[HW, 16], [W, 8], [1, W]]),
                            in_=bass.AP(stg.tensor, k * W,
                                        [[NC2 * 4 * W, 128], [1, W]]),
                        )

            # software pipeline: loads one super ahead, back-stage one behind
            emit_loads(0)
            prev = None
            for b in range(NB):
                if b + 1 < NB:
                    emit_loads(b + 1)
                if prev is not None:
                    emit_back(b - 1, prev)
                prev = emit_front(b)
            emit_back(NB - 1, prev)

    nc.compile()
    return nc


def _make_in_maps(inputs: dict) -> list:
    fL = np.asarray(inputs["fL"], dtype=np.float32).astype(ml_dtypes.bfloat16)
    fR = np.asarray(inputs["fR"], dtype=np.float32).astype(ml_dtypes.bfloat16)
    fL = np.ascontiguousarray(fL)
    fR = np.ascontiguousarray(fR)
    return [{"fL": fL[k], "fR": fR[k]} for k in range(N_CORES)]


def kernel(fL: np.ndarray, fR: np.ndarray) -> np.ndarray:
    if "nc" not in _cache:
        _cache["nc"] = _build()
    nc = _cache["nc"]

    in_maps = _make_in_maps({"fL": fL, "fR": fR})
    res = run_bass_kernel_spmd(nc, in_maps, core_ids=list(range(N_CORES)))
    out = np.stack(
        [res.results[k]["out"].astype(np.float32) for k in range(N_CORES)],
        axis=0,
    )
    return out


if __name__ == "__main__":
    rng = np.random.default_rng(0)
    a = rng.standard_normal((N_CORES, C, H, W)).astype(np.float32)
    b = rng.standard_normal((N_CORES, C, H, W)).astype(np.float32)
    o = kernel(a, b)
    print("kernel ran, output shape", o.shape)
